# revision 14
# baseline (speedup 1.0000x reference)
"""Trainium2 Bass kernel for ConditionalCrossAttnDiT.

Strategy: pure data parallel over batch (B=32 -> 4 images per NeuronCore x 8 cores).
Device computes the token-level network (patch embed, 6 transformer blocks, final
projection) with feature-major activations (features on partitions, tokens on the
free dim) and bf16 matmuls on the PE. Host (numpy) does cheap O(B*D) work:
patchify/unpatchify, timestep embeddings + adaLN conditioning vectors, weight
transposition/packing.
"""
import sys

sys.path.insert(0, "/opt/trn_rl_repo")

import numpy as np
import ml_dtypes

import concourse.bass as bass  # noqa: F401
import concourse.mybir as mybir
import concourse.tile as tile
from concourse import bacc
from concourse.bass_utils import run_bass_kernel_spmd
from concourse.masks import make_identity

BFNP = ml_dtypes.bfloat16
FP = mybir.dt.float32
BF = mybir.dt.bfloat16
AF = mybir.ActivationFunctionType
ALU = mybir.AluOpType

# Model dims
B, C, IMG, P, D, DEPTH, H = 32, 3, 256, 16, 384, 6, 6
NP_ = IMG // P
T = NP_ * NP_        # 256 tokens per image
MLP = 4 * D          # 1536
NFREQ = 256
PDIM = P * P * C     # 768
DH = D // H          # 64
SQD = float(np.sqrt(D))

N_CORES = 8
NB = B // N_CORES    # 4 images per core
N = NB * T           # 1024 tokens per core
KD = D // 128        # 3 feature chunks
N_CROSS = DEPTH // 2

# weight specs: key -> (KT, M) for [KT,128,M] bf16 DRAM tensors (per block)
WSPEC = {
    "sa_qk": (3, 768),
    "sa_v": (3, 384),
    "sa_out": (3, 384),
    "mlp1": (3, 1536),
    "mlp2": (12, 384),
    "k_w": (3, 384),
    "v_w": (3, 384),
    "ca_qk": (3, 768),
    "ca_v": (3, 384),
    "ca_out": (3, 384),
    "po": (3, 384),
}
BLOCK_KEYS = ("sa_qk", "sa_v", "sa_out", "mlp1", "mlp2")
CROSS_KEYS = ("k_w", "v_w", "ca_qk", "ca_v", "ca_out", "po")


def _build_colmap():
    cols = {}
    nv = 0

    def add(name, n):
        nonlocal nv
        cols[name] = nv
        nv += n

    for bi in range(DEPTH):
        cross = bi < N_CROSS
        kinds = ("self", "cross", "mlp") if cross else ("self", "mlp")
        for kind in kinds:
            for img in range(NB):
                add(f"b{bi}_{kind}_a1_{img}", KD)
                add(f"b{bi}_{kind}_a2_{img}", KD)
                add(f"b{bi}_{kind}_a0_{img}", KD)
        add(f"b{bi}_sa_qk_b", 6)
        add(f"b{bi}_sa_v_b", 3)
        add(f"b{bi}_sa_out_b", 3)
        add(f"b{bi}_mlp1_b", 12)
        add(f"b{bi}_mlp2_b", 3)
        if cross:
            add(f"b{bi}_k_b", 3)
            add(f"b{bi}_v_b", 3)
            add(f"b{bi}_ca_qk_b", 6)
            add(f"b{bi}_ca_v_b", 3)
            add(f"b{bi}_ca_out_b", 3)
            add(f"b{bi}_po_b", 3)
    add("emb_x_b", 3)
    add("emb_c_b", 3)
    add("pos_x", KD * T)
    add("pos_c", KD * T)
    for img in range(NB):
        add(f"fin_a1_{img}", KD)
        add(f"fin_a0_{img}", KD)
    return cols, nv


COLS, NV = _build_colmap()


# ---------------------------------------------------------------------------
# Device program
# ---------------------------------------------------------------------------
def build_nc():
    nc = bacc.Bacc("TRN2", target_bir_lowering=False, debug=False, num_devices=N_CORES)

    xp_d = nc.dram_tensor("xp", [6, 128, N], BF, kind="ExternalInput")
    cp_d = nc.dram_tensor("cp", [6, 128, N], BF, kind="ExternalInput")
    vec_d = nc.dram_tensor("vecs", [128, NV], FP, kind="ExternalInput")
    embx_d = nc.dram_tensor("emb_x", [6, 128, D], BF, kind="ExternalInput")
    embc_d = nc.dram_tensor("emb_c", [6, 128, D], BF, kind="ExternalInput")
    proj_d = nc.dram_tensor("proj", [3, 128, PDIM], BF, kind="ExternalInput")
    wd = {}
    for bi in range(DEPTH):
        keys = BLOCK_KEYS + (CROSS_KEYS if bi < N_CROSS else ())
        for key in keys:
            kt, m = WSPEC[key]
            wd[(bi, key)] = nc.dram_tensor(
                f"w{bi}_{key}", [kt, 128, m], BF, kind="ExternalInput")
    out_d = nc.dram_tensor("out", [N, PDIM], FP, kind="ExternalOutput")

    with tile.TileContext(nc) as tc:
        _body(nc, tc, xp_d, cp_d, vec_d, embx_d, embc_d, proj_d, wd, out_d)

    nc.compile()
    return nc


def _body(nc, tc, xp_d, cp_d, vec_d, embx_d, embc_d, proj_d, wd, out_d):
    from contextlib import ExitStack

    ctx = ExitStack()
    const = ctx.enter_context(tc.tile_pool(name="const", bufs=1))
    persist = ctx.enter_context(tc.tile_pool(name="persist", bufs=1))
    acts = ctx.enter_context(tc.tile_pool(name="acts", bufs=1))
    small = ctx.enter_context(tc.tile_pool(name="small", bufs=2))
    wpool = ctx.enter_context(tc.tile_pool(name="wpool", bufs=2))
    ps3 = ctx.enter_context(tc.tile_pool(name="ps3", bufs=4, space="PSUM"))
    ps2 = ctx.enter_context(tc.tile_pool(name="ps2", bufs=3, space="PSUM"))
    ps1 = ctx.enter_context(tc.tile_pool(name="ps1", bufs=1, space="PSUM"))

    # constants
    ident = const.tile([128, 128], BF)
    make_identity(nc, ident)
    ones_kd_bf = const.tile([128, 1], BF)
    nc.vector.memset(ones_kd_bf[:], 1.0)
    ones_kd_f = const.tile([128, 1], FP)
    nc.vector.memset(ones_kd_f[:], 1.0)
    ones_k1_f = const.tile([1, 128], FP)
    nc.vector.memset(ones_k1_f[:], 1.0)
    ones_k1_bf = const.tile([1, 128], BF)
    nc.vector.memset(ones_k1_bf[:], 1.0)

    vecs = persist.tile([128, NV], FP)
    nc.sync.dma_start(vecs[:], vec_d.ap())

    def vcol(name, off=0, n=1, p0=0, pn=128):
        c = COLS[name] + off
        return vecs[p0 : p0 + pn, c : c + n]

    # persistent activations (feature-major: [128 part, KD chunk, N tokens])
    X = persist.tile([128, KD, N], FP)
    Xb = persist.tile([128, KD, N], BF)
    Cb = persist.tile([128, KD, N], BF)

    # ---------------- embedding ----------------
    with tc.tile_pool(name="embed", bufs=1) as epool:
        for (img_d, emb_d, bname, posname, fp32_dst) in (
            (xp_d, embx_d, "emb_x_b", "pos_x", True),
            (cp_d, embc_d, "emb_c_b", "pos_c", False),
        ):
            src = epool.tile([128, 6, N], BF, tag="ep_src", name="ep_src")
            nc.sync.dma_start(src[:], img_d.ap().rearrange("k p n -> p k n"))
            wsb = epool.tile([128, 6, D], BF, tag="ep_w", name="ep_w")
            nc.sync.dma_start(wsb[:], emb_d.ap().rearrange("k p m -> p k m"))
            for mc in range(KD):
                for nh in range(2):
                    ps = ps3.tile([128, 512], FP, tag="mmS", name="emb_ps")
                    for k in range(6):
                        nc.tensor.matmul(
                            ps[:],
                            wsb[:, k, mc * 128 : (mc + 1) * 128],
                            src[:, k, nh * 512 : (nh + 1) * 512],
                            start=(k == 0),
                            stop=(k == 5),
                        )
                    dst = X if fp32_dst else Cb
                    for j in range(2):
                        img = nh * 2 + j
                        nc.vector.scalar_tensor_tensor(
                            dst[:, mc, img * T : (img + 1) * T],
                            ps[:, j * T : (j + 1) * T],
                            vcol(bname, mc),
                            vcol(posname, mc * T, T),
                            op0=ALU.add,
                            op1=ALU.add,
                        )
                    if fp32_dst:
                        nc.scalar.activation(
                            Xb[:, mc, nh * 512 : (nh + 1) * 512],
                            X[:, mc, nh * 512 : (nh + 1) * 512], AF.Copy)

    # ---------------- helpers ----------------
    def load_w(bi, key):
        kt, m = WSPEC[key]
        w = wpool.tile([128, kt, m], BF, tag=key, name=f"w{bi}_{key}",
                       bufs=2 if key in ("sa_qk", "sa_v", "sa_out", "mlp1") else 1)
        nc.sync.dma_start(w[:], wd[(bi, key)].ap().rearrange("k p m -> p k m"))
        return w

    def linear_fmajor(w, kt, mt, rhs, evict, m0=0, nh_outer=False):
        """evict(psum, mc, nh) over psum[mc,nh] = sum_k w[:,k,(m0+mc)*128+...].T @ rhs."""
        pairs = ([(mc, nh) for nh in range(2) for mc in range(mt)]
                 if nh_outer else [(mc, nh) for mc in range(mt) for nh in range(2)])
        for mc, nh in pairs:
            if True:
                ps = ps3.tile([128, 512], FP, tag="mmS", name="lin_ps")
                for k in range(kt):
                    nc.tensor.matmul(
                        ps[:],
                        w[:, k, (m0 + mc) * 128 : (m0 + mc + 1) * 128],
                        rhs[:, k, nh * 512 : (nh + 1) * 512],
                        start=(k == 0),
                        stop=(k == kt - 1),
                    )
                evict(ps, mc, nh)

    def evict_bias(dst, bias_name, boff=0, engine="vec"):
        def ev(ps, mc, nh):
            sl = dst[:, mc, nh * 512 : (nh + 1) * 512]
            if engine == "act":
                nc.scalar.activation(sl, ps[:], AF.Identity,
                                     bias=vcol(bias_name, boff + mc))
            else:
                nc.vector.tensor_scalar(
                    sl, ps[:], vcol(bias_name, boff + mc), None, op0=ALU.add)
        return ev

    def evict_residual(bias_name):
        """X += psum + bias ; refresh Xb."""
        def ev(ps, mc, nh):
            xsl = X[:, mc, nh * 512 : (nh + 1) * 512]
            nc.vector.scalar_tensor_tensor(
                xsl, ps[:], vcol(bias_name, mc), xsl, op0=ALU.add, op1=ALU.add)
            nc.scalar.activation(
                Xb[:, mc, nh * 512 : (nh + 1) * 512], xsl, AF.Copy)
        return ev

    def adaln_modulate(bi, kind):
        """returns q = a1*Xb + a2*(r@Xb) + a0 per image; r = 1/||x_tok|| (bf16)."""
        q = acts.tile([128, KD, N], BF, tag="q_mod", bufs=1, name=f"q_{bi}_{kind}")
        for nh in range(2):
            ssq = ps3.tile([1, 512], FP, tag="mmS", name="ssq")
            for k in range(KD):
                sq = small.tile([128, 512], BF, tag="sq", name="sq")
                nc.vector.tensor_tensor(
                    sq[:], Xb[:, k, nh * 512 : (nh + 1) * 512],
                    Xb[:, k, nh * 512 : (nh + 1) * 512], op=ALU.mult)
                nc.tensor.matmul(ssq[:], ones_kd_bf[:], sq[:],
                                 start=(k == 0), stop=(k == KD - 1))
            rr = small.tile([1, 512], BF, tag="rr", name="rr")
            nc.scalar.activation(rr[:], ssq[:], AF.Abs_reciprocal_sqrt)
            rb = ps1.tile([128, 512], FP, tag="rb", name="rb")
            nc.tensor.matmul(rb[:], ones_k1_bf[:], rr[:], start=True, stop=True)
            rb_sb = small.tile([128, 512], BF, tag="rbsb", name="rbsb")
            nc.vector.tensor_copy(rb_sb[:], rb[:])
            for k in range(KD):
                tmid = small.tile([128, 512], BF, tag="tmid", name="tmid")
                nc.vector.tensor_tensor(
                    tmid[:], Xb[:, k, nh * 512 : (nh + 1) * 512], rb_sb[:], op=ALU.mult)
                for j in range(2):
                    img = nh * 2 + j
                    u = small.tile([128, 256], BF, tag="umod", name="umod")
                    nc.vector.tensor_scalar(
                        u[:], tmid[:, j * T : (j + 1) * T],
                        vcol(f"b{bi}_{kind}_a2_{img}", k),
                        vcol(f"b{bi}_{kind}_a0_{img}", k),
                        op0=ALU.mult, op1=ALU.add)
                    nc.vector.scalar_tensor_tensor(
                        q[:, k, img * T : (img + 1) * T],
                        Xb[:, k, img * T : (img + 1) * T],
                        vcol(f"b{bi}_{kind}_a1_{img}", k),
                        u[:],
                        op0=ALU.mult, op1=ALU.add)
        return q

    def attention(qp, kp, vtok, Osb):
        """qp,kp f-major [128,KD,N]; vtok token-major head-structured
        [128, 2NB, H*(DH+1)] with a ones column per head (Z rides the AV matmul);
        Osb f-major out (v-bias folded into out-proj bias on host)."""
        Otok = acts.tile([128, 2 * NB, D], BF, tag="Otok", bufs=1, name="Otok")
        for img in range(NB):
            for h in range(H):
                p0 = (h % 2) * 64
                kdh = h // 2
                # S^T[kt, qt] both chunks in one psum tile, one exp
                ST = ps3.tile([128, 2, T], FP, tag="mmS", name="attST")
                for kti in range(2):
                    nc.tensor.matmul(
                        ST[:, kti, :],
                        kp[p0 : p0 + 64, kdh,
                           img * T + kti * 128 : img * T + (kti + 1) * 128],
                        qp[p0 : p0 + 64, kdh, img * T : (img + 1) * T],
                        start=True, stop=True)
                E = small.tile([128, 2, T], BF, tag="attE", bufs=6, name="attE")
                nc.scalar.activation(E[:], ST[:], AF.Exp)
                # O_tok[qt, dh(+Z)] = sum_kt E^T[kt,qt-chunk] . vtok[kt, head cols]
                Ops = ps2.tile([128, 2, DH + 1], FP, tag="attO", name="attO")
                for c in range(2):
                    for kti in range(2):
                        nc.tensor.matmul(
                            Ops[:, c, :],
                            E[:, kti, c * 128 : (c + 1) * 128],
                            vtok[:, img * 2 + kti,
                                 h * (DH + 1) : (h + 1) * (DH + 1)],
                            start=(kti == 0), stop=(kti == 1))
                rz = small.tile([128, 2], FP, tag="attRZ", bufs=6, name="attRZ")
                nc.vector.reciprocal(rz[:], Ops[:, :, DH])
                for c in range(2):
                    nc.vector.tensor_scalar(
                        Otok[:, img * 2 + c, h * DH : (h + 1) * DH],
                        Ops[:, c, :DH], rz[:, c : c + 1], None, op0=ALU.mult)
        # transpose O_tok back to feature-major Osb
        for ch in range(2 * NB):
            OT = ps2.tile([128, KD, 128], BF, tag="attO", name="attOT")
            for f in range(KD):
                nc.tensor.transpose(
                    OT[:, f, :], Otok[:, ch, f * 128 : (f + 1) * 128], ident[:])
            nc.scalar.activation(Osb[:, :, ch * 128 : (ch + 1) * 128], OT[:], AF.Copy)

    def linear_tokmajor(dst, lhs_src, w):
        """dst[:, mc, head-structured] = sum_k lhs_src[:,k,mc*128:+128].T @ w[:,k,:],
        interleaving a ones column after each head's DH columns."""
        dst4 = dst.rearrange("p a (h u) -> p a h u", u=DH + 1)
        nc.gpsimd.memset(dst4[:, :, :, DH : DH + 1], 1.0)
        for mc in range(2 * NB):
            ps = ps3.tile([128, D], FP, tag="mmS", name="tok_ps")
            for k in range(KD):
                nc.tensor.matmul(
                    ps[:],
                    lhs_src[:, k, mc * 128 : (mc + 1) * 128],
                    w[:, k, :],
                    start=(k == 0), stop=(k == KD - 1))
            nc.scalar.activation(
                dst4[:, mc, :, 0:DH],
                ps[:].rearrange("p (h u) -> p h u", u=DH), AF.Copy)

    # ---------------- blocks ----------------
    for bi in range(DEPTH):
        cross = bi < N_CROSS

        # --- self attention (cross pre-projections emitted first: they only
        # depend on Cb, so the scheduler can fill self-attention PE gaps) ---
        w_qk = load_w(bi, "sa_qk")
        w_v = load_w(bi, "sa_v")
        w_o = load_w(bi, "sa_out")
        if cross:
            w_k = load_w(bi, "k_w")
            w_v2 = load_w(bi, "v_w")
            kpre = acts.tile([128, KD, N], BF, tag="kpre", bufs=1, name=f"kpre{bi}")
            vpre = acts.tile([128, KD, N], BF, tag="vpre", bufs=1, name=f"vpre{bi}")
            linear_fmajor(w_k, KD, KD, Cb, evict_bias(kpre, f"b{bi}_k_b"), nh_outer=True)
            linear_fmajor(w_v2, KD, KD, Cb, evict_bias(vpre, f"b{bi}_v_b"), nh_outer=True)
        q = adaln_modulate(bi, "self")
        qp = acts.tile([128, KD, N], BF, tag="qp", bufs=1, name=f"qp{bi}")
        kp = acts.tile([128, KD, N], BF, tag="kp", bufs=1, name=f"kp{bi}")
        vtok = acts.tile([128, 2 * NB, H * (DH + 1)], BF, tag="vtok", bufs=1, name=f"vtok{bi}")
        Osb = acts.tile([128, KD, N], BF, tag="Osb", bufs=1, name=f"Osb{bi}")
        linear_fmajor(w_qk, KD, KD, q, evict_bias(qp, f"b{bi}_sa_qk_b", engine="act"), nh_outer=True)
        linear_fmajor(w_qk, KD, KD, q, evict_bias(kp, f"b{bi}_sa_qk_b", boff=KD, engine="act"), m0=KD, nh_outer=True)
        linear_tokmajor(vtok, q, w_v)
        attention(qp, kp, vtok, Osb)
        linear_fmajor(w_o, KD, KD, Osb, evict_residual(f"b{bi}_sa_out_b"), nh_outer=True)

        # --- cross attention ---
        if cross:
            w_cqk = load_w(bi, "ca_qk")
            w_cv = load_w(bi, "ca_v")
            w_co = load_w(bi, "ca_out")
            w_po = load_w(bi, "po")
            q = adaln_modulate(bi, "cross")
            qp = acts.tile([128, KD, N], BF, tag="qp", bufs=1, name=f"cqp{bi}")
            kp = acts.tile([128, KD, N], BF, tag="kp", bufs=1, name=f"ckp{bi}")
            vtok = acts.tile([128, 2 * NB, H * (DH + 1)], BF, tag="vtok", bufs=1, name=f"cvtok{bi}")
            Osb = acts.tile([128, KD, N], BF, tag="Osb", bufs=1, name=f"cOsb{bi}")
            linear_fmajor(w_cqk, KD, KD, q, evict_bias(qp, f"b{bi}_ca_qk_b", engine="act"), nh_outer=True)
            linear_fmajor(w_cqk, KD, KD, kpre,
                          evict_bias(kp, f"b{bi}_ca_qk_b", boff=KD, engine="act"),
                          m0=KD, nh_outer=True)
            linear_tokmajor(vtok, vpre, w_cv)
            attention(qp, kp, vtok, Osb)
            o1 = acts.tile([128, KD, N], BF, tag="kpre", bufs=1, name=f"o1_{bi}")
            linear_fmajor(w_co, KD, KD, Osb, evict_bias(o1, f"b{bi}_ca_out_b"), nh_outer=True)
            linear_fmajor(w_po, KD, KD, o1, evict_residual(f"b{bi}_po_b"), nh_outer=True)

        # --- mlp ---
        w1 = load_w(bi, "mlp1")
        w2 = load_w(bi, "mlp2")
        q = adaln_modulate(bi, "mlp")
        for nh in range(2):
            hsb = acts.tile([128, 12, 512], BF, tag="hsb", bufs=1, name=f"hsb{bi}_{nh}")
            for mc in range(12):
                ps = ps3.tile([128, 512], FP, tag="mmS", name="h_ps")
                for k in range(KD):
                    nc.tensor.matmul(
                        ps[:],
                        w1[:, k, mc * 128 : (mc + 1) * 128],
                        q[:, k, nh * 512 : (nh + 1) * 512],
                        start=(k == 0), stop=(k == KD - 1))
                nc.scalar.activation(hsb[:, mc, :], ps[:], AF.Gelu,
                                     bias=vcol(f"b{bi}_mlp1_b", mc))
            for mc in range(KD):
                ps = ps3.tile([128, 512], FP, tag="mmS", name="h2_ps")
                for k in range(12):
                    nc.tensor.matmul(
                        ps[:], w2[:, k, mc * 128 : (mc + 1) * 128], hsb[:, k, :],
                        start=(k == 0), stop=(k == 11))
                xsl = X[:, mc, nh * 512 : (nh + 1) * 512]
                nc.vector.scalar_tensor_tensor(
                    xsl, ps[:], vcol(f"b{bi}_mlp2_b", mc), xsl,
                    op0=ALU.add, op1=ALU.add)
                nc.scalar.activation(
                    Xb[:, mc, nh * 512 : (nh + 1) * 512], xsl, AF.Copy)

    # ---------------- final ----------------
    with tc.tile_pool(name="finpool", bufs=1) as fpool:
        wpr = fpool.tile([128, 3, PDIM], BF)
        nc.sync.dma_start(wpr[:], proj_d.ap().rearrange("k p m -> p k m"))
        y = acts.tile([128, KD, N], BF, tag="q_mod", bufs=1, name="y_fin")
        for nh in range(2):
            ssq = ps3.tile([1, 512], FP, tag="mmS", name="fin_ssq")
            for k in range(KD):
                sqf = small.tile([128, 512], FP, tag="sqf", name="sqf")
                nc.vector.tensor_tensor(
                    sqf[:], X[:, k, nh * 512 : (nh + 1) * 512],
                    X[:, k, nh * 512 : (nh + 1) * 512], op=ALU.mult)
                nc.tensor.matmul(ssq[:], ones_kd_f[:], sqf[:],
                                 start=(k == 0), stop=(k == KD - 1))
            rnorm = small.tile([1, 512], FP, tag="rnorm", name="fin_rnorm")
            nc.scalar.activation(rnorm[:], ssq[:], AF.Sqrt)
            rr = small.tile([1, 512], FP, tag="rr", name="fin_rr")
            nc.vector.reciprocal(rr[:], rnorm[:])
            rb = ps1.tile([128, 512], FP, tag="rb", name="fin_rb")
            nc.tensor.matmul(rb[:], ones_k1_f[:], rr[:], start=True, stop=True)
            for k in range(KD):
                tmid = small.tile([128, 512], FP, tag="tmidf", name="tmidf")
                nc.vector.tensor_tensor(
                    tmid[:], X[:, k, nh * 512 : (nh + 1) * 512], rb[:], op=ALU.mult)
                for j in range(2):
                    img = nh * 2 + j
                    nc.vector.tensor_scalar(
                        y[:, k, img * T : (img + 1) * T],
                        tmid[:, j * T : (j + 1) * T],
                        vcol(f"fin_a1_{img}", k),
                        vcol(f"fin_a0_{img}", k),
                        op0=ALU.mult, op1=ALU.add)
        for mc in range(2 * NB):
            ost = small.tile([128, PDIM], FP, tag="ost", name="ost")
            for half in range(2):
                ps = ps3.tile([128, 384], FP, tag="mmS", name="fin_ps")
                for k in range(KD):
                    nc.tensor.matmul(
                        ps[:],
                        y[:, k, mc * 128 : (mc + 1) * 128],
                        wpr[:, k, half * 384 : (half + 1) * 384],
                        start=(k == 0), stop=(k == KD - 1))
                nc.vector.tensor_copy(ost[:, half * 384 : (half + 1) * 384], ps[:])
            nc.sync.dma_start(out_d[mc * 128 : (mc + 1) * 128, :], ost[:])

    ctx.close()


# ---------------------------------------------------------------------------
# Host side
# ---------------------------------------------------------------------------
def _silu(x):
    return x / (1.0 + np.exp(-x))


def _host_precompute(t, r, params):
    def temb_mlp(tv, p):
        half = NFREQ // 2
        freqs = np.exp(-np.log(10000.0) * np.arange(half, dtype=np.float32) / half)
        args = (np.asarray(tv, np.float32) * 1000.0)[:, None] * freqs[None, :]
        e = np.concatenate([np.cos(args), np.sin(args)], axis=-1).astype(np.float32)
        h = _silu(e @ np.asarray(p["w1"], np.float32).T + np.asarray(p["b1"], np.float32))
        return h @ np.asarray(p["w2"], np.float32).T + np.asarray(p["b2"], np.float32)

    temb = temb_mlp(t, params["t_emb"]) + temb_mlp(r, params["r_emb"])
    c = _silu(temb)
    blocks = []
    for bp in params["blocks"]:
        bd = {}
        for kind, wk, bk, gk in (
            ("self", "ada_self_w", "ada_self_b", "ada_self_g"),
            ("cross", "ada_cross_w", "ada_cross_b", "ada_cross_g"),
            ("mlp", "ada_mlp_w", "ada_mlp_b", "ada_mlp_g"),
        ):
            v = c @ np.asarray(bp[wk], np.float32).T + np.asarray(bp[bk], np.float32)
            sh, sc, gate = v[:, :D], v[:, D : 2 * D], v[:, 2 * D :]
            g = float(np.asarray(bp[gk]).reshape(-1)[0])
            a1 = (1.0 + sc).astype(np.float32)
            a2 = (gate * SQD * g * a1).astype(np.float32)
            a0 = sh.astype(np.float32)
            bd[kind] = (a1, a2, a0)
        blocks.append(bd)
    f = params["final"]
    v = c @ np.asarray(f["mod_w"], np.float32).T + np.asarray(f["mod_b"], np.float32)
    sh, sc = v[:, :D], v[:, D:]
    gf = float(np.asarray(f["g"]).reshape(-1)[0])
    fa1 = (SQD * gf * (1.0 + sc)).astype(np.float32)
    fa0 = sh.astype(np.float32)
    return blocks, (fa1, fa0)


def _patchify(img):
    b = np.asarray(img).shape[0]
    return (
        np.asarray(img, np.float32)
        .reshape(b, C, NP_, P, NP_, P)
        .transpose(0, 2, 4, 1, 3, 5)
        .reshape(b, T, PDIM)
    )


def _chunk_cols(vec):
    """[dim] -> [128, dim//128] feature-chunk columns."""
    v = np.asarray(vec, np.float32).reshape(-1)
    return v.reshape(v.shape[0] // 128, 128).T


def _wt_pack(w, q_scale_cols=0):
    """w [dout, din] -> [din//128, 128, dout] bf16 (transposed, chunked)."""
    wt = np.asarray(w, np.float32).T.copy()
    if q_scale_cols:
        wt[:, :q_scale_cols] = wt[:, :q_scale_cols] / 8.0
    din, dout = wt.shape
    return np.ascontiguousarray(wt.reshape(din // 128, 128, dout)).astype(BFNP)


_NC_CACHE = {}


def _get_nc():
    if "nc" not in _NC_CACHE:
        _NC_CACHE["nc"] = build_nc()
    return _NC_CACHE["nc"]


def build_in_maps(x, t, r, cond_image, params):
    xp = _patchify(x)
    cp = _patchify(cond_image)
    blocks_ada, (fa1, fa0) = _host_precompute(t, r, params)

    shared = {}
    for bi, bp in enumerate(params["blocks"]):
        sa_in = np.asarray(bp["sa_in_w"], np.float32)     # [3D, D]
        shared[f"w{bi}_sa_qk"] = _wt_pack(sa_in[: 2 * D], q_scale_cols=D)
        shared[f"w{bi}_sa_v"] = _wt_pack(sa_in[2 * D :])
        shared[f"w{bi}_sa_out"] = _wt_pack(bp["sa_out_w"])
        shared[f"w{bi}_mlp1"] = _wt_pack(bp["mlp_w1"])
        shared[f"w{bi}_mlp2"] = _wt_pack(bp["mlp_w2"])
        if bi < N_CROSS:
            ca_in = np.asarray(bp["ca_in_w"], np.float32)
            shared[f"w{bi}_k_w"] = _wt_pack(bp["k_w"])
            shared[f"w{bi}_v_w"] = _wt_pack(bp["v_w"])
            shared[f"w{bi}_ca_qk"] = _wt_pack(ca_in[: 2 * D], q_scale_cols=D)
            shared[f"w{bi}_ca_v"] = _wt_pack(ca_in[2 * D :])
            shared[f"w{bi}_ca_out"] = _wt_pack(bp["ca_out_w"])
            shared[f"w{bi}_po"] = _wt_pack(bp["po_w"])
    shared["emb_x"] = _wt_pack(params["x_emb_w"])
    shared["emb_c"] = _wt_pack(params["c_emb_w"])
    shared["proj"] = _wt_pack(params["final"]["proj_w"])

    in_maps = []
    for core in range(N_CORES):
        sl = slice(core * NB, (core + 1) * NB)
        vec = np.zeros((128, NV), np.float32)

        def setv(name, mat, off=0):
            c = COLS[name] + off
            m = np.asarray(mat, np.float32)
            if m.ndim == 1:
                m = m[:, None]
            vec[:, c : c + m.shape[1]] = m

        for bi, bp in enumerate(params["blocks"]):
            cross = bi < N_CROSS
            kinds = ("self", "cross", "mlp") if cross else ("self", "mlp")
            for kind in kinds:
                a1, a2, a0 = blocks_ada[bi][kind]
                for j in range(NB):
                    gi = core * NB + j
                    setv(f"b{bi}_{kind}_a1_{j}", _chunk_cols(a1[gi]))
                    setv(f"b{bi}_{kind}_a2_{j}", _chunk_cols(a2[gi]))
                    setv(f"b{bi}_{kind}_a0_{j}", _chunk_cols(a0[gi]))
            qkb = np.concatenate(
                [np.asarray(bp["sa_in_b"], np.float32)[:D] / 8.0,
                 np.asarray(bp["sa_in_b"], np.float32)[D : 2 * D]])
            setv(f"b{bi}_sa_qk_b", _chunk_cols(qkb))
            # v-bias folded through the out projection (O has no bias on device)
            sa_bv = np.asarray(bp["sa_in_b"], np.float32)[2 * D :]
            sa_out_b = (np.asarray(bp["sa_out_b"], np.float32)
                        + np.asarray(bp["sa_out_w"], np.float32) @ sa_bv)
            setv(f"b{bi}_sa_out_b", _chunk_cols(sa_out_b))
            setv(f"b{bi}_mlp1_b", _chunk_cols(bp["mlp_b1"]))
            setv(f"b{bi}_mlp2_b", _chunk_cols(bp["mlp_b2"]))
            if cross:
                cqkb = np.concatenate(
                    [np.asarray(bp["ca_in_b"], np.float32)[:D] / 8.0,
                     np.asarray(bp["ca_in_b"], np.float32)[D : 2 * D]])
                setv(f"b{bi}_k_b", _chunk_cols(bp["k_b"]))
                setv(f"b{bi}_v_b", _chunk_cols(bp["v_b"]))
                setv(f"b{bi}_ca_qk_b", _chunk_cols(cqkb))
                ca_bv = np.asarray(bp["ca_in_b"], np.float32)[2 * D :]
                ca_out_b = (np.asarray(bp["ca_out_b"], np.float32)
                            + np.asarray(bp["ca_out_w"], np.float32) @ ca_bv)
                setv(f"b{bi}_ca_out_b", _chunk_cols(ca_out_b))
                setv(f"b{bi}_po_b", _chunk_cols(bp["po_b"]))
        setv("emb_x_b", _chunk_cols(params["x_emb_b"]))
        setv("emb_c_b", _chunk_cols(params["c_emb_b"]))
        pos_x = np.asarray(params["pos_x"], np.float32)[0]  # [T, D]
        pos_c = np.asarray(params["pos_c"], np.float32)[0]
        for k in range(KD):
            setv("pos_x", pos_x[:, k * 128 : (k + 1) * 128].T, off=k * T)
            setv("pos_c", pos_c[:, k * 128 : (k + 1) * 128].T, off=k * T)
        for j in range(NB):
            gi = core * NB + j
            setv(f"fin_a1_{j}", _chunk_cols(fa1[gi]))
            setv(f"fin_a0_{j}", _chunk_cols(fa0[gi]))

        xpc = np.ascontiguousarray(
            xp[sl].reshape(N, PDIM).T.reshape(6, 128, N)).astype(BFNP)
        cpc = np.ascontiguousarray(
            cp[sl].reshape(N, PDIM).T.reshape(6, 128, N)).astype(BFNP)
        im = dict(shared)
        im["xp"] = xpc
        im["cp"] = cpc
        im["vecs"] = vec
        in_maps.append(im)
    return in_maps


def assemble_output(results, params):
    proj_b = np.asarray(params["final"]["proj_b"], np.float32)
    out_tok = np.concatenate(
        [np.asarray(results[i]["out"]).reshape(NB, T, PDIM) for i in range(N_CORES)],
        axis=0,
    ) + proj_b[None, None, :]
    return (
        out_tok.reshape(B, NP_, NP_, P, P, C)
        .transpose(0, 5, 1, 3, 2, 4)
        .reshape(B, C, IMG, IMG)
        .astype(np.float32)
    )


def kernel(x, t, r, cond_image, params):
    nc = _get_nc()
    in_maps = build_in_maps(x, t, r, cond_image, params)
    res = run_bass_kernel_spmd(nc, in_maps, list(range(N_CORES)))
    return assemble_output(res.results, params)


# revision 16
# speedup vs baseline: 1.0189x; 1.0189x over previous
"""Trainium2 Bass kernel for ConditionalCrossAttnDiT.

Strategy: pure data parallel over batch (B=32 -> 4 images per NeuronCore x 8 cores).
Device computes the token-level network (patch embed, 6 transformer blocks, final
projection) with feature-major activations (features on partitions, tokens on the
free dim) and bf16 matmuls on the PE. Host (numpy) does cheap O(B*D) work:
patchify/unpatchify, timestep embeddings + adaLN conditioning vectors, weight
transposition/packing.
"""
import sys

sys.path.insert(0, "/opt/trn_rl_repo")

import numpy as np
import ml_dtypes

import concourse.bass as bass  # noqa: F401
import concourse.mybir as mybir
import concourse.tile as tile
from concourse import bacc
from concourse.bass_utils import run_bass_kernel_spmd
from concourse.masks import make_identity

BFNP = ml_dtypes.bfloat16
FP = mybir.dt.float32
BF = mybir.dt.bfloat16
AF = mybir.ActivationFunctionType
ALU = mybir.AluOpType

# Model dims
B, C, IMG, P, D, DEPTH, H = 32, 3, 256, 16, 384, 6, 6
NP_ = IMG // P
T = NP_ * NP_        # 256 tokens per image
MLP = 4 * D          # 1536
NFREQ = 256
PDIM = P * P * C     # 768
DH = D // H          # 64
SQD = float(np.sqrt(D))

N_CORES = 8
NB = B // N_CORES    # 4 images per core
N = NB * T           # 1024 tokens per core
KD = D // 128        # 3 feature chunks
N_CROSS = DEPTH // 2

# weight specs: key -> (KT, M) for [KT,128,M] bf16 DRAM tensors (per block)
WSPEC = {
    "sa_qk": (3, 768),
    "sa_v": (3, 384),
    "sa_out": (3, 384),
    "mlp1": (3, 1536),
    "mlp2": (12, 384),
    "k_w": (3, 384),
    "v_w": (3, 384),
    "ca_qk": (3, 768),
    "ca_v": (3, 384),
    "ca_out": (3, 384),
    "po": (3, 384),
}
BLOCK_KEYS = ("sa_qk", "sa_v", "sa_out", "mlp1", "mlp2")
CROSS_KEYS = ("k_w", "v_w", "ca_qk", "ca_v", "ca_out", "po")


def _build_colmap():
    cols = {}
    nv = 0

    def add(name, n):
        nonlocal nv
        cols[name] = nv
        nv += n

    for bi in range(DEPTH):
        cross = bi < N_CROSS
        kinds = ("self", "cross", "mlp") if cross else ("self", "mlp")
        for kind in kinds:
            for img in range(NB):
                add(f"b{bi}_{kind}_a1_{img}", KD)
                add(f"b{bi}_{kind}_a2_{img}", KD)
                add(f"b{bi}_{kind}_a0_{img}", KD)
        add(f"b{bi}_sa_qk_b", 6)
        add(f"b{bi}_sa_v_b", 3)
        add(f"b{bi}_sa_out_b", 3)
        add(f"b{bi}_mlp1_b", 12)
        add(f"b{bi}_mlp2_b", 3)
        if cross:
            add(f"b{bi}_k_b", 3)
            add(f"b{bi}_v_b", 3)
            add(f"b{bi}_ca_qk_b", 6)
            add(f"b{bi}_ca_v_b", 3)
            add(f"b{bi}_ca_out_b", 3)
            add(f"b{bi}_po_b", 3)
    add("emb_x_b", 3)
    add("emb_c_b", 3)
    add("pos_x", KD * T)
    add("pos_c", KD * T)
    for img in range(NB):
        add(f"fin_a1_{img}", KD)
        add(f"fin_a0_{img}", KD)
    return cols, nv


COLS, NV = _build_colmap()


# ---------------------------------------------------------------------------
# Device program
# ---------------------------------------------------------------------------
def build_nc():
    nc = bacc.Bacc("TRN2", target_bir_lowering=False, debug=False, num_devices=N_CORES)

    xp_d = nc.dram_tensor("xp", [6, 128, N], BF, kind="ExternalInput")
    cp_d = nc.dram_tensor("cp", [6, 128, N], BF, kind="ExternalInput")
    vec_d = nc.dram_tensor("vecs", [128, NV], FP, kind="ExternalInput")
    embx_d = nc.dram_tensor("emb_x", [6, 128, D], BF, kind="ExternalInput")
    embc_d = nc.dram_tensor("emb_c", [6, 128, D], BF, kind="ExternalInput")
    proj_d = nc.dram_tensor("proj", [3, 128, PDIM], BF, kind="ExternalInput")
    wd = {}
    for bi in range(DEPTH):
        keys = BLOCK_KEYS + (CROSS_KEYS if bi < N_CROSS else ())
        for key in keys:
            kt, m = WSPEC[key]
            wd[(bi, key)] = nc.dram_tensor(
                f"w{bi}_{key}", [kt, 128, m], BF, kind="ExternalInput")
    out_d = nc.dram_tensor("out", [N, PDIM], FP, kind="ExternalOutput")

    with tile.TileContext(nc) as tc:
        _body(nc, tc, xp_d, cp_d, vec_d, embx_d, embc_d, proj_d, wd, out_d)

    nc.compile()
    return nc


def _body(nc, tc, xp_d, cp_d, vec_d, embx_d, embc_d, proj_d, wd, out_d):
    from contextlib import ExitStack

    ctx = ExitStack()
    const = ctx.enter_context(tc.tile_pool(name="const", bufs=1))
    persist = ctx.enter_context(tc.tile_pool(name="persist", bufs=1))
    acts = ctx.enter_context(tc.tile_pool(name="acts", bufs=1))
    small = ctx.enter_context(tc.tile_pool(name="small", bufs=2))
    wpool = ctx.enter_context(tc.tile_pool(name="wpool", bufs=2))
    ps3 = ctx.enter_context(tc.tile_pool(name="ps3", bufs=4, space="PSUM"))
    ps2 = ctx.enter_context(tc.tile_pool(name="ps2", bufs=3, space="PSUM"))
    ps1 = ctx.enter_context(tc.tile_pool(name="ps1", bufs=1, space="PSUM"))

    # constants
    ident = const.tile([128, 128], BF)
    make_identity(nc, ident)
    ones_kd_bf = const.tile([128, 1], BF)
    nc.vector.memset(ones_kd_bf[:], 1.0)
    ones_kd_f = const.tile([128, 1], FP)
    nc.vector.memset(ones_kd_f[:], 1.0)
    ones_k1_f = const.tile([1, 128], FP)
    nc.vector.memset(ones_k1_f[:], 1.0)
    ones_k1_bf = const.tile([1, 128], BF)
    nc.vector.memset(ones_k1_bf[:], 1.0)

    vecs = persist.tile([128, NV], FP)
    nc.sync.dma_start(vecs[:], vec_d.ap())

    def vcol(name, off=0, n=1, p0=0, pn=128):
        c = COLS[name] + off
        return vecs[p0 : p0 + pn, c : c + n]

    # persistent activations (feature-major: [128 part, KD chunk, N tokens])
    X = persist.tile([128, KD, N], FP)
    Xb = persist.tile([128, KD, N], BF)
    Cb = persist.tile([128, KD, N], BF)

    # ---------------- embedding ----------------
    with tc.tile_pool(name="embed", bufs=1) as epool:
        for (img_d, emb_d, bname, posname, fp32_dst) in (
            (xp_d, embx_d, "emb_x_b", "pos_x", True),
            (cp_d, embc_d, "emb_c_b", "pos_c", False),
        ):
            src = epool.tile([128, 6, N], BF, tag="ep_src", name="ep_src")
            nc.sync.dma_start(src[:], img_d.ap().rearrange("k p n -> p k n"))
            wsb = epool.tile([128, 6, D], BF, tag="ep_w", name="ep_w")
            nc.sync.dma_start(wsb[:], emb_d.ap().rearrange("k p m -> p k m"))
            for mc in range(KD):
                for nh in range(2):
                    ps = ps3.tile([128, 512], FP, tag="mmS", name="emb_ps")
                    for k in range(6):
                        nc.tensor.matmul(
                            ps[:],
                            wsb[:, k, mc * 128 : (mc + 1) * 128],
                            src[:, k, nh * 512 : (nh + 1) * 512],
                            start=(k == 0),
                            stop=(k == 5),
                        )
                    dst = X if fp32_dst else Cb
                    for j in range(2):
                        img = nh * 2 + j
                        nc.vector.scalar_tensor_tensor(
                            dst[:, mc, img * T : (img + 1) * T],
                            ps[:, j * T : (j + 1) * T],
                            vcol(bname, mc),
                            vcol(posname, mc * T, T),
                            op0=ALU.add,
                            op1=ALU.add,
                        )
                    if fp32_dst:
                        nc.scalar.activation(
                            Xb[:, mc, nh * 512 : (nh + 1) * 512],
                            X[:, mc, nh * 512 : (nh + 1) * 512], AF.Copy)

    # ---------------- helpers ----------------
    def load_w(bi, key):
        kt, m = WSPEC[key]
        w = wpool.tile([128, kt, m], BF, tag=key, name=f"w{bi}_{key}",
                       bufs=2 if key in ("sa_qk", "sa_v", "sa_out") else 1)
        nc.sync.dma_start(w[:], wd[(bi, key)].ap().rearrange("k p m -> p k m"))
        return w

    def linear_fmajor(w, kt, mt, rhs, evict, m0=0, nh_outer=False):
        """evict(psum, mc, nh) over psum[mc,nh] = sum_k w[:,k,(m0+mc)*128+...].T @ rhs."""
        pairs = ([(mc, nh) for nh in range(2) for mc in range(mt)]
                 if nh_outer else [(mc, nh) for mc in range(mt) for nh in range(2)])
        for mc, nh in pairs:
            if True:
                ps = ps3.tile([128, 512], FP, tag="mmS", name="lin_ps")
                for k in range(kt):
                    nc.tensor.matmul(
                        ps[:],
                        w[:, k, (m0 + mc) * 128 : (m0 + mc + 1) * 128],
                        rhs[:, k, nh * 512 : (nh + 1) * 512],
                        start=(k == 0),
                        stop=(k == kt - 1),
                    )
                evict(ps, mc, nh)

    def evict_bias(dst, bias_name, boff=0, engine="vec"):
        def ev(ps, mc, nh):
            sl = dst[:, mc, nh * 512 : (nh + 1) * 512]
            if engine == "act":
                nc.scalar.activation(sl, ps[:], AF.Identity,
                                     bias=vcol(bias_name, boff + mc))
            else:
                nc.vector.tensor_scalar(
                    sl, ps[:], vcol(bias_name, boff + mc), None, op0=ALU.add)
        return ev

    def evict_residual(bias_name, sq_out=None):
        """X += psum + bias ; refresh Xb; optionally pre-compute rms squares."""
        def ev(ps, mc, nh):
            xsl = X[:, mc, nh * 512 : (nh + 1) * 512]
            nc.vector.scalar_tensor_tensor(
                xsl, ps[:], vcol(bias_name, mc), xsl, op0=ALU.add, op1=ALU.add)
            xbsl = Xb[:, mc, nh * 512 : (nh + 1) * 512]
            nc.scalar.activation(xbsl, xsl, AF.Copy)
            if sq_out is not None:
                sq = small.tile([128, 512], BF, tag="sq", bufs=8, name="sqpre")
                nc.vector.tensor_tensor(sq[:], xbsl, xbsl, op=ALU.mult)
                sq_out[(nh, mc)] = sq
        return ev

    def adaln_modulate(bi, kind, sq_pre=None):
        """returns q = a1*Xb + a2*(r@Xb) + a0 per image; r = 1/||x_tok|| (bf16)."""
        q = acts.tile([128, KD, N], BF, tag="q_mod", bufs=1, name=f"q_{bi}_{kind}")
        for nh in range(2):
            ssq = ps3.tile([1, 512], FP, tag="mmS", name="ssq")
            for k in range(KD):
                if sq_pre is not None:
                    sq = sq_pre[(nh, k)]
                else:
                    sq = small.tile([128, 512], BF, tag="sq", bufs=8, name="sq")
                    nc.vector.tensor_tensor(
                        sq[:], Xb[:, k, nh * 512 : (nh + 1) * 512],
                        Xb[:, k, nh * 512 : (nh + 1) * 512], op=ALU.mult)
                nc.tensor.matmul(ssq[:], ones_kd_bf[:], sq[:],
                                 start=(k == 0), stop=(k == KD - 1))
            rr = small.tile([1, 512], BF, tag="rr", name="rr")
            nc.scalar.activation(rr[:], ssq[:], AF.Abs_reciprocal_sqrt)
            rb = ps1.tile([128, 512], FP, tag="rb", name="rb")
            nc.tensor.matmul(rb[:], ones_k1_bf[:], rr[:], start=True, stop=True)
            rb_sb = small.tile([128, 512], BF, tag="rbsb", name="rbsb")
            nc.vector.tensor_copy(rb_sb[:], rb[:])
            for k in range(KD):
                tmid = small.tile([128, 512], BF, tag="tmid", name="tmid")
                nc.vector.tensor_tensor(
                    tmid[:], Xb[:, k, nh * 512 : (nh + 1) * 512], rb_sb[:], op=ALU.mult)
                for j in range(2):
                    img = nh * 2 + j
                    u = small.tile([128, 256], BF, tag="umod", name="umod")
                    nc.vector.tensor_scalar(
                        u[:], tmid[:, j * T : (j + 1) * T],
                        vcol(f"b{bi}_{kind}_a2_{img}", k),
                        vcol(f"b{bi}_{kind}_a0_{img}", k),
                        op0=ALU.mult, op1=ALU.add)
                    nc.vector.scalar_tensor_tensor(
                        q[:, k, img * T : (img + 1) * T],
                        Xb[:, k, img * T : (img + 1) * T],
                        vcol(f"b{bi}_{kind}_a1_{img}", k),
                        u[:],
                        op0=ALU.mult, op1=ALU.add)
        return q

    def attention(qp, kp, vtok, Osb):
        """qp,kp f-major [128,KD,N]; vtok token-major head-structured
        [128, 2NB, H*(DH+1)] with a ones column per head (Z rides the AV matmul);
        Osb f-major out (v-bias folded into out-proj bias on host)."""
        Otok = acts.tile([128, 2 * NB, D], BF, tag="Otok", bufs=1, name="Otok")
        for img in range(NB):
            for h in range(H):
                p0 = (h % 2) * 64
                kdh = h // 2
                # S^T[kt, qt] both chunks in one psum tile, one exp
                ST = ps3.tile([128, 2, T], FP, tag="mmS", name="attST")
                for kti in range(2):
                    nc.tensor.matmul(
                        ST[:, kti, :],
                        kp[p0 : p0 + 64, kdh,
                           img * T + kti * 128 : img * T + (kti + 1) * 128],
                        qp[p0 : p0 + 64, kdh, img * T : (img + 1) * T],
                        start=True, stop=True)
                E = small.tile([128, 2, T], BF, tag="attE", bufs=4, name="attE")
                nc.scalar.activation(E[:], ST[:], AF.Exp)
                # O_tok[qt, dh(+Z)] = sum_kt E^T[kt,qt-chunk] . vtok[kt, head cols]
                Ops = ps2.tile([128, 2, DH + 1], FP, tag="attO", name="attO")
                for c in range(2):
                    for kti in range(2):
                        nc.tensor.matmul(
                            Ops[:, c, :],
                            E[:, kti, c * 128 : (c + 1) * 128],
                            vtok[:, img * 2 + kti,
                                 h * (DH + 1) : (h + 1) * (DH + 1)],
                            start=(kti == 0), stop=(kti == 1))
                rz = small.tile([128, 2], FP, tag="attRZ", bufs=4, name="attRZ")
                nc.vector.reciprocal(rz[:], Ops[:, :, DH])
                for c in range(2):
                    nc.vector.tensor_scalar(
                        Otok[:, img * 2 + c, h * DH : (h + 1) * DH],
                        Ops[:, c, :DH], rz[:, c : c + 1], None, op0=ALU.mult)
        # transpose O_tok back to feature-major Osb
        for ch in range(2 * NB):
            OT = ps2.tile([128, KD, 128], BF, tag="attO", name="attOT")
            for f in range(KD):
                nc.tensor.transpose(
                    OT[:, f, :], Otok[:, ch, f * 128 : (f + 1) * 128], ident[:])
            nc.scalar.activation(Osb[:, :, ch * 128 : (ch + 1) * 128], OT[:], AF.Copy)

    def linear_tokmajor(dst, lhs_src, w):
        """dst[:, mc, head-structured] = sum_k lhs_src[:,k,mc*128:+128].T @ w[:,k,:],
        interleaving a ones column after each head's DH columns."""
        dst4 = dst.rearrange("p a (h u) -> p a h u", u=DH + 1)
        nc.gpsimd.memset(dst4[:, :, :, DH : DH + 1], 1.0)
        for mc in range(2 * NB):
            ps = ps3.tile([128, D], FP, tag="mmS", name="tok_ps")
            for k in range(KD):
                nc.tensor.matmul(
                    ps[:],
                    lhs_src[:, k, mc * 128 : (mc + 1) * 128],
                    w[:, k, :],
                    start=(k == 0), stop=(k == KD - 1))
            nc.scalar.activation(
                dst4[:, mc, :, 0:DH],
                ps[:].rearrange("p (h u) -> p h u", u=DH), AF.Copy)

    # ---------------- blocks ----------------
    sq_prev = {}
    for bi in range(DEPTH):
        cross = bi < N_CROSS

        # --- self attention (cross pre-projections emitted first: they only
        # depend on Cb, so the scheduler can fill self-attention PE gaps) ---
        w_qk = load_w(bi, "sa_qk")
        w_v = load_w(bi, "sa_v")
        w_o = load_w(bi, "sa_out")
        if cross:
            w_k = load_w(bi, "k_w")
            w_v2 = load_w(bi, "v_w")
            kpre = acts.tile([128, KD, N], BF, tag="kpre", bufs=1, name=f"kpre{bi}")
            vpre = acts.tile([128, KD, N], BF, tag="vpre", bufs=1, name=f"vpre{bi}")
            linear_fmajor(w_k, KD, KD, Cb, evict_bias(kpre, f"b{bi}_k_b"), nh_outer=True)
            linear_fmajor(w_v2, KD, KD, Cb, evict_bias(vpre, f"b{bi}_v_b"), nh_outer=True)
        q = adaln_modulate(bi, "self", sq_pre=sq_prev.get(bi))
        qp = acts.tile([128, KD, N], BF, tag="qp", bufs=1, name=f"qp{bi}")
        kp = acts.tile([128, KD, N], BF, tag="kp", bufs=1, name=f"kp{bi}")
        vtok = acts.tile([128, 2 * NB, H * (DH + 1)], BF, tag="vtok", bufs=1, name=f"vtok{bi}")
        Osb = acts.tile([128, KD, N], BF, tag="Osb", bufs=1, name=f"Osb{bi}")
        linear_fmajor(w_qk, KD, KD, q, evict_bias(qp, f"b{bi}_sa_qk_b", engine="act"), nh_outer=True)
        linear_fmajor(w_qk, KD, KD, q, evict_bias(kp, f"b{bi}_sa_qk_b", boff=KD, engine="act"), m0=KD, nh_outer=True)
        linear_tokmajor(vtok, q, w_v)
        attention(qp, kp, vtok, Osb)
        sq_next = {}
        linear_fmajor(w_o, KD, KD, Osb,
                      evict_residual(f"b{bi}_sa_out_b", sq_out=sq_next), nh_outer=True)

        # --- cross attention ---
        if cross:
            w_cqk = load_w(bi, "ca_qk")
            w_cv = load_w(bi, "ca_v")
            w_co = load_w(bi, "ca_out")
            w_po = load_w(bi, "po")
            q = adaln_modulate(bi, "cross", sq_pre=sq_next)
            qp = acts.tile([128, KD, N], BF, tag="qp", bufs=1, name=f"cqp{bi}")
            kp = acts.tile([128, KD, N], BF, tag="kp", bufs=1, name=f"ckp{bi}")
            vtok = acts.tile([128, 2 * NB, H * (DH + 1)], BF, tag="vtok", bufs=1, name=f"cvtok{bi}")
            Osb = acts.tile([128, KD, N], BF, tag="Osb", bufs=1, name=f"cOsb{bi}")
            linear_fmajor(w_cqk, KD, KD, q, evict_bias(qp, f"b{bi}_ca_qk_b", engine="act"), nh_outer=True)
            linear_fmajor(w_cqk, KD, KD, kpre,
                          evict_bias(kp, f"b{bi}_ca_qk_b", boff=KD, engine="act"),
                          m0=KD, nh_outer=True)
            linear_tokmajor(vtok, vpre, w_cv)
            attention(qp, kp, vtok, Osb)
            o1 = acts.tile([128, KD, N], BF, tag="kpre", bufs=1, name=f"o1_{bi}")
            linear_fmajor(w_co, KD, KD, Osb, evict_bias(o1, f"b{bi}_ca_out_b"), nh_outer=True)
            sq_next = {}
            linear_fmajor(w_po, KD, KD, o1,
                          evict_residual(f"b{bi}_po_b", sq_out=sq_next), nh_outer=True)

        # --- mlp ---
        sq_self = {} if bi + 1 < DEPTH else None
        w1 = load_w(bi, "mlp1")
        w2 = load_w(bi, "mlp2")
        q = adaln_modulate(bi, "mlp", sq_pre=sq_next)
        for nh in range(2):
            hsb = acts.tile([128, 12, 512], BF, tag="hsb", bufs=1, name=f"hsb{bi}_{nh}")
            for mc in range(12):
                ps = ps3.tile([128, 512], FP, tag="mmS", name="h_ps")
                for k in range(KD):
                    nc.tensor.matmul(
                        ps[:],
                        w1[:, k, mc * 128 : (mc + 1) * 128],
                        q[:, k, nh * 512 : (nh + 1) * 512],
                        start=(k == 0), stop=(k == KD - 1))
                nc.scalar.activation(hsb[:, mc, :], ps[:], AF.Gelu,
                                     bias=vcol(f"b{bi}_mlp1_b", mc))
            for mc in range(KD):
                ps = ps3.tile([128, 512], FP, tag="mmS", name="h2_ps")
                for k in range(12):
                    nc.tensor.matmul(
                        ps[:], w2[:, k, mc * 128 : (mc + 1) * 128], hsb[:, k, :],
                        start=(k == 0), stop=(k == 11))
                xsl = X[:, mc, nh * 512 : (nh + 1) * 512]
                nc.vector.scalar_tensor_tensor(
                    xsl, ps[:], vcol(f"b{bi}_mlp2_b", mc), xsl,
                    op0=ALU.add, op1=ALU.add)
                xbsl = Xb[:, mc, nh * 512 : (nh + 1) * 512]
                nc.scalar.activation(xbsl, xsl, AF.Copy)
                if sq_self is not None:
                    sqt = small.tile([128, 512], BF, tag="sq", bufs=8, name="sqpre2")
                    nc.vector.tensor_tensor(sqt[:], xbsl, xbsl, op=ALU.mult)
                    sq_self[(nh, mc)] = sqt
                    sq_prev[bi + 1] = sq_self

    # ---------------- final ----------------
    with tc.tile_pool(name="finpool", bufs=1) as fpool:
        wpr = fpool.tile([128, 3, PDIM], BF)
        nc.sync.dma_start(wpr[:], proj_d.ap().rearrange("k p m -> p k m"))
        y = acts.tile([128, KD, N], BF, tag="q_mod", bufs=1, name="y_fin")
        for nh in range(2):
            ssq = ps3.tile([1, 512], FP, tag="mmS", name="fin_ssq")
            for k in range(KD):
                sqf = small.tile([128, 512], FP, tag="sqf", name="sqf")
                nc.vector.tensor_tensor(
                    sqf[:], X[:, k, nh * 512 : (nh + 1) * 512],
                    X[:, k, nh * 512 : (nh + 1) * 512], op=ALU.mult)
                nc.tensor.matmul(ssq[:], ones_kd_f[:], sqf[:],
                                 start=(k == 0), stop=(k == KD - 1))
            rnorm = small.tile([1, 512], FP, tag="rnorm", name="fin_rnorm")
            nc.scalar.activation(rnorm[:], ssq[:], AF.Sqrt)
            rr = small.tile([1, 512], FP, tag="rr", name="fin_rr")
            nc.vector.reciprocal(rr[:], rnorm[:])
            rb = ps1.tile([128, 512], FP, tag="rb", name="fin_rb")
            nc.tensor.matmul(rb[:], ones_k1_f[:], rr[:], start=True, stop=True)
            for k in range(KD):
                tmid = small.tile([128, 512], FP, tag="tmidf", name="tmidf")
                nc.vector.tensor_tensor(
                    tmid[:], X[:, k, nh * 512 : (nh + 1) * 512], rb[:], op=ALU.mult)
                for j in range(2):
                    img = nh * 2 + j
                    nc.vector.tensor_scalar(
                        y[:, k, img * T : (img + 1) * T],
                        tmid[:, j * T : (j + 1) * T],
                        vcol(f"fin_a1_{img}", k),
                        vcol(f"fin_a0_{img}", k),
                        op0=ALU.mult, op1=ALU.add)
        for mc in range(2 * NB):
            ost = small.tile([128, PDIM], FP, tag="ost", name="ost")
            for half in range(2):
                ps = ps3.tile([128, 384], FP, tag="mmS", name="fin_ps")
                for k in range(KD):
                    nc.tensor.matmul(
                        ps[:],
                        y[:, k, mc * 128 : (mc + 1) * 128],
                        wpr[:, k, half * 384 : (half + 1) * 384],
                        start=(k == 0), stop=(k == KD - 1))
                nc.vector.tensor_copy(ost[:, half * 384 : (half + 1) * 384], ps[:])
            nc.sync.dma_start(out_d[mc * 128 : (mc + 1) * 128, :], ost[:])

    ctx.close()


# ---------------------------------------------------------------------------
# Host side
# ---------------------------------------------------------------------------
def _silu(x):
    return x / (1.0 + np.exp(-x))


def _host_precompute(t, r, params):
    def temb_mlp(tv, p):
        half = NFREQ // 2
        freqs = np.exp(-np.log(10000.0) * np.arange(half, dtype=np.float32) / half)
        args = (np.asarray(tv, np.float32) * 1000.0)[:, None] * freqs[None, :]
        e = np.concatenate([np.cos(args), np.sin(args)], axis=-1).astype(np.float32)
        h = _silu(e @ np.asarray(p["w1"], np.float32).T + np.asarray(p["b1"], np.float32))
        return h @ np.asarray(p["w2"], np.float32).T + np.asarray(p["b2"], np.float32)

    temb = temb_mlp(t, params["t_emb"]) + temb_mlp(r, params["r_emb"])
    c = _silu(temb)
    blocks = []
    for bp in params["blocks"]:
        bd = {}
        for kind, wk, bk, gk in (
            ("self", "ada_self_w", "ada_self_b", "ada_self_g"),
            ("cross", "ada_cross_w", "ada_cross_b", "ada_cross_g"),
            ("mlp", "ada_mlp_w", "ada_mlp_b", "ada_mlp_g"),
        ):
            v = c @ np.asarray(bp[wk], np.float32).T + np.asarray(bp[bk], np.float32)
            sh, sc, gate = v[:, :D], v[:, D : 2 * D], v[:, 2 * D :]
            g = float(np.asarray(bp[gk]).reshape(-1)[0])
            a1 = (1.0 + sc).astype(np.float32)
            a2 = (gate * SQD * g * a1).astype(np.float32)
            a0 = sh.astype(np.float32)
            bd[kind] = (a1, a2, a0)
        blocks.append(bd)
    f = params["final"]
    v = c @ np.asarray(f["mod_w"], np.float32).T + np.asarray(f["mod_b"], np.float32)
    sh, sc = v[:, :D], v[:, D:]
    gf = float(np.asarray(f["g"]).reshape(-1)[0])
    fa1 = (SQD * gf * (1.0 + sc)).astype(np.float32)
    fa0 = sh.astype(np.float32)
    return blocks, (fa1, fa0)


def _patchify(img):
    b = np.asarray(img).shape[0]
    return (
        np.asarray(img, np.float32)
        .reshape(b, C, NP_, P, NP_, P)
        .transpose(0, 2, 4, 1, 3, 5)
        .reshape(b, T, PDIM)
    )


def _chunk_cols(vec):
    """[dim] -> [128, dim//128] feature-chunk columns."""
    v = np.asarray(vec, np.float32).reshape(-1)
    return v.reshape(v.shape[0] // 128, 128).T


def _wt_pack(w, q_scale_cols=0):
    """w [dout, din] -> [din//128, 128, dout] bf16 (transposed, chunked)."""
    wt = np.asarray(w, np.float32).T.copy()
    if q_scale_cols:
        wt[:, :q_scale_cols] = wt[:, :q_scale_cols] / 8.0
    din, dout = wt.shape
    return np.ascontiguousarray(wt.reshape(din // 128, 128, dout)).astype(BFNP)


_NC_CACHE = {}


def _get_nc():
    if "nc" not in _NC_CACHE:
        _NC_CACHE["nc"] = build_nc()
    return _NC_CACHE["nc"]


def build_in_maps(x, t, r, cond_image, params):
    xp = _patchify(x)
    cp = _patchify(cond_image)
    blocks_ada, (fa1, fa0) = _host_precompute(t, r, params)

    shared = {}
    for bi, bp in enumerate(params["blocks"]):
        sa_in = np.asarray(bp["sa_in_w"], np.float32)     # [3D, D]
        shared[f"w{bi}_sa_qk"] = _wt_pack(sa_in[: 2 * D], q_scale_cols=D)
        shared[f"w{bi}_sa_v"] = _wt_pack(sa_in[2 * D :])
        shared[f"w{bi}_sa_out"] = _wt_pack(bp["sa_out_w"])
        shared[f"w{bi}_mlp1"] = _wt_pack(bp["mlp_w1"])
        shared[f"w{bi}_mlp2"] = _wt_pack(bp["mlp_w2"])
        if bi < N_CROSS:
            ca_in = np.asarray(bp["ca_in_w"], np.float32)
            shared[f"w{bi}_k_w"] = _wt_pack(bp["k_w"])
            shared[f"w{bi}_v_w"] = _wt_pack(bp["v_w"])
            shared[f"w{bi}_ca_qk"] = _wt_pack(ca_in[: 2 * D], q_scale_cols=D)
            shared[f"w{bi}_ca_v"] = _wt_pack(ca_in[2 * D :])
            shared[f"w{bi}_ca_out"] = _wt_pack(bp["ca_out_w"])
            shared[f"w{bi}_po"] = _wt_pack(bp["po_w"])
    shared["emb_x"] = _wt_pack(params["x_emb_w"])
    shared["emb_c"] = _wt_pack(params["c_emb_w"])
    shared["proj"] = _wt_pack(params["final"]["proj_w"])

    in_maps = []
    for core in range(N_CORES):
        sl = slice(core * NB, (core + 1) * NB)
        vec = np.zeros((128, NV), np.float32)

        def setv(name, mat, off=0):
            c = COLS[name] + off
            m = np.asarray(mat, np.float32)
            if m.ndim == 1:
                m = m[:, None]
            vec[:, c : c + m.shape[1]] = m

        for bi, bp in enumerate(params["blocks"]):
            cross = bi < N_CROSS
            kinds = ("self", "cross", "mlp") if cross else ("self", "mlp")
            for kind in kinds:
                a1, a2, a0 = blocks_ada[bi][kind]
                for j in range(NB):
                    gi = core * NB + j
                    setv(f"b{bi}_{kind}_a1_{j}", _chunk_cols(a1[gi]))
                    setv(f"b{bi}_{kind}_a2_{j}", _chunk_cols(a2[gi]))
                    setv(f"b{bi}_{kind}_a0_{j}", _chunk_cols(a0[gi]))
            qkb = np.concatenate(
                [np.asarray(bp["sa_in_b"], np.float32)[:D] / 8.0,
                 np.asarray(bp["sa_in_b"], np.float32)[D : 2 * D]])
            setv(f"b{bi}_sa_qk_b", _chunk_cols(qkb))
            # v-bias folded through the out projection (O has no bias on device)
            sa_bv = np.asarray(bp["sa_in_b"], np.float32)[2 * D :]
            sa_out_b = (np.asarray(bp["sa_out_b"], np.float32)
                        + np.asarray(bp["sa_out_w"], np.float32) @ sa_bv)
            setv(f"b{bi}_sa_out_b", _chunk_cols(sa_out_b))
            setv(f"b{bi}_mlp1_b", _chunk_cols(bp["mlp_b1"]))
            setv(f"b{bi}_mlp2_b", _chunk_cols(bp["mlp_b2"]))
            if cross:
                cqkb = np.concatenate(
                    [np.asarray(bp["ca_in_b"], np.float32)[:D] / 8.0,
                     np.asarray(bp["ca_in_b"], np.float32)[D : 2 * D]])
                setv(f"b{bi}_k_b", _chunk_cols(bp["k_b"]))
                setv(f"b{bi}_v_b", _chunk_cols(bp["v_b"]))
                setv(f"b{bi}_ca_qk_b", _chunk_cols(cqkb))
                ca_bv = np.asarray(bp["ca_in_b"], np.float32)[2 * D :]
                ca_out_b = (np.asarray(bp["ca_out_b"], np.float32)
                            + np.asarray(bp["ca_out_w"], np.float32) @ ca_bv)
                setv(f"b{bi}_ca_out_b", _chunk_cols(ca_out_b))
                setv(f"b{bi}_po_b", _chunk_cols(bp["po_b"]))
        setv("emb_x_b", _chunk_cols(params["x_emb_b"]))
        setv("emb_c_b", _chunk_cols(params["c_emb_b"]))
        pos_x = np.asarray(params["pos_x"], np.float32)[0]  # [T, D]
        pos_c = np.asarray(params["pos_c"], np.float32)[0]
        for k in range(KD):
            setv("pos_x", pos_x[:, k * 128 : (k + 1) * 128].T, off=k * T)
            setv("pos_c", pos_c[:, k * 128 : (k + 1) * 128].T, off=k * T)
        for j in range(NB):
            gi = core * NB + j
            setv(f"fin_a1_{j}", _chunk_cols(fa1[gi]))
            setv(f"fin_a0_{j}", _chunk_cols(fa0[gi]))

        xpc = np.ascontiguousarray(
            xp[sl].reshape(N, PDIM).T.reshape(6, 128, N)).astype(BFNP)
        cpc = np.ascontiguousarray(
            cp[sl].reshape(N, PDIM).T.reshape(6, 128, N)).astype(BFNP)
        im = dict(shared)
        im["xp"] = xpc
        im["cp"] = cpc
        im["vecs"] = vec
        in_maps.append(im)
    return in_maps


def assemble_output(results, params):
    proj_b = np.asarray(params["final"]["proj_b"], np.float32)
    out_tok = np.concatenate(
        [np.asarray(results[i]["out"]).reshape(NB, T, PDIM) for i in range(N_CORES)],
        axis=0,
    ) + proj_b[None, None, :]
    return (
        out_tok.reshape(B, NP_, NP_, P, P, C)
        .transpose(0, 5, 1, 3, 2, 4)
        .reshape(B, C, IMG, IMG)
        .astype(np.float32)
    )


def kernel(x, t, r, cond_image, params):
    nc = _get_nc()
    in_maps = build_in_maps(x, t, r, cond_image, params)
    res = run_bass_kernel_spmd(nc, in_maps, list(range(N_CORES)))
    return assemble_output(res.results, params)
